# revision 13
# baseline (speedup 1.0000x reference)
"""Trainium2 Bass kernel for nn_AutoregressiveRegression (LSTM warmup + autoregressive decode).

Problem (per reference):
  B=512, T=128, F=4, U=1024, out_steps=32
  - warmup: LSTM over x[:, t, :] for t in 0..T-1 (h0=c0=0)
  - pred0 = h @ dense_w + dense_b
  - decode: 31 more LSTM steps feeding pred back as the input
  - output: [B, out_steps, 4]

Strategy (8 NeuronCores, data-parallel over batch, 64 rows/core):
  - All weights resident in SBUF; zero steady-state DMA, zero collectives.
  - The recurrence is PE-streaming-bound: each step streams the whole
    rec_kernel through the array as the moving operand (M=64 batch).
  - Hybrid precision, exploiting the LSTM forget-gate's geometric decay of
    injected noise (validated by numpy simulation; measured HW rel err
    4.0e-3 vs the 2e-2 gate):
      * steps t < NSTEPS-K_EXACT: fp8e4(e4m3) weights+h with DoubleRow
        matmuls (K=256 contraction per pass, 32 MMs/step vs 64).
      * last K_EXACT=40 steps (incl. all decode): bf16 weights+h; the
        bf16 noise floor dominates the final error.
    (On this walrus build DoubleRow does not reach its nominal 2x moving-
    operand rate - fp8 steps measure near bf16-step speed - but the halved
    matmul count plus shared-stationary pairing still wins. Measured
    2.46 ms total vs 2.80 ms baseline, repetition-slope method.)
  - Matmul emission order (pair, k, jj-bank) keeps consecutive matmuls on
    the same stationary operand, amortizing PE weight loads; the next
    step's input matmuls are issued early to fill the gate-math tail.
  - Weights are pre-scaled x64 host-side (fp8 dynamic-range centering);
    the descale is folded into the activation instructions (scale=1/64).
  - Weight columns gate-interleaved host-side per 128-unit slice j:
    [i_j | f_j | o_j | g_j]; j-chunk PAIRS share one [64, 2, 512] PSUM
    tile (2 banks) so all gate math runs as 3D-AP instructions covering
    both chunks -> half the ACT/DVE instruction overhead.
  - h is re-transposed each step on the PE; the PSUM->SBUF evacuation
    doubles as the fp8/bf16 downcast for the next step's stationary.
  - Decode feeds pred back via a bf16 staging block; the f32 copy of
    pred is the output staging.
"""

import os
from contextlib import ExitStack

import numpy as np

B_FULL = 512
T_WARM = 128
N_CORES = 8
B_LOC = B_FULL // N_CORES  # 64
U = 1024
NF = 4
K_EXACT = 40
W_SCALE = 64.0

# bf16 constant block column layout (elements per partition)
_KB0 = 0                      # kernel+bias rows 0:5: [5, 4096]
_XT0 = _KB0 + 4 * U           # x^T + ones row: [5, T*b]
_DW0 = _XT0 + T_WARM * B_LOC  # dense_w chunk-major: [128, 32]
_DX0 = _DW0 + 32              # decode input staging: [5, S*b] (row 4 = ones)


def _cb_cols(S):
    return _DX0 + S * B_LOC


# f32r constant block layout
_ID0 = 0                      # identity [128, 128]
_IA0 = _ID0 + 128             # output staging [4, S*b]


def _db_col(S):
    return _IA0 + S * B_LOC


def _cr_cols(S):
    return _db_col(S) + 1


def _build_program(S, reps=1):
    """Build the per-core Bass program (identical on all cores; data differs).

    reps > 1 wraps the whole computation (including load DMAs) in a hardware
    For_i loop — used only for timing (slope over reps isolates on-device
    exec time from the axon RPC noise)."""
    import concourse.mybir as mybir
    import concourse.tile as tile
    from concourse import bacc

    F32 = mybir.dt.float32
    F32R = mybir.dt.float32r
    BF16 = mybir.dt.bfloat16
    FP8 = mybir.dt.float8e4
    AF = mybir.ActivationFunctionType
    DR = mybir.MatmulPerfMode.DoubleRow

    T = T_WARM
    b = B_LOC
    NSTEPS = T + S - 1       # 159 recurrent steps
    T_FP8 = NSTEPS - K_EXACT  # steps t < T_FP8 use fp8 DoubleRow

    nc = bacc.Bacc("TRN2", target_bir_lowering=False, debug=False)

    c8_d = nc.dram_tensor("c8", [128, 8 * 4 * U], FP8, kind="ExternalInput").ap()
    cb_d = nc.dram_tensor("cb", [128, 8 * 4 * U + _cb_cols(S)], BF16,
                          kind="ExternalInput").ap()
    cr_d = nc.dram_tensor("cr", [128, _cr_cols(S)], F32R,
                          kind="ExternalInput").ap()
    outp_d = nc.dram_tensor("outp", [4, S * b], F32R, kind="ExternalOutput").ap()

    with tile.TileContext(nc) as tc, ExitStack() as ctx:
        singles = ctx.enter_context(tc.tile_pool(name="singles", bufs=1))
        h8pool = ctx.enter_context(tc.tile_pool(name="h8pool", bufs=2))
        hbpool = ctx.enter_context(tc.tile_pool(name="hbpool", bufs=2))
        hpool = ctx.enter_context(tc.tile_pool(name="hpool", bufs=2))
        gpool = ctx.enter_context(tc.tile_pool(name="gpool", bufs=2))
        zpool = ctx.enter_context(tc.tile_pool(name="zpool", bufs=3, space="PSUM"))
        tppool = ctx.enter_context(tc.tile_pool(name="tppool", bufs=2, space="PSUM"))

        rep_ctx = tc.For_i(0, reps, 1) if reps > 1 else None
        if rep_ctx is not None:
            rep_ctx.__enter__()

        # DMA order matters: cstb (x/kernel, needed at t=0) and the fp8
        # weights (needed at t=1) first; the bf16 weights are not consumed
        # until step T_FP8 (~1.1 ms in), so their 8 MB load hides behind
        # the fp8-region compute.
        w8 = singles.tile([128, 8, 4 * U], FP8, tag="w8")
        wb = singles.tile([128, 8, 4 * U], BF16, tag="wb")
        cstb = singles.tile([128, _cb_cols(S)], BF16, tag="cstb")
        nc.sync.dma_start(out=cstb, in_=cb_d[:, 8 * 4 * U :])
        cstr = singles.tile([128, _cr_cols(S)], F32R, tag="cstr")
        nc.sync.dma_start(out=cstr, in_=cr_d)
        for k in range(8):
            nc.sync.dma_start(out=w8[:, k, :],
                              in_=c8_d[:, k * 4 * U : (k + 1) * 4 * U])
        for k in range(8):
            nc.sync.dma_start(out=wb[:, k, :],
                              in_=cb_d[:, k * 4 * U : (k + 1) * 4 * U])

        kb_sb = cstb[0:5, _KB0 : _KB0 + 4 * U]
        xt_sb = cstb[0:5, _XT0 : _XT0 + T * b]
        dw_sb = cstb[:, _DW0 : _DW0 + 32]
        dxt = cstb[0:5, _DX0 : _DX0 + S * b]
        ident64 = cstr[0:64, _ID0 : _ID0 + 64]
        in_all = cstr[0:4, _IA0 : _IA0 + S * b]
        db_sb = singles.tile([4, 1], F32, tag="db")
        nc.gpsimd.dma_start(out=db_sb, in_=cr_d[0:4, _db_col(S) : _db_col(S) + 1])

        c_sb = singles.tile([64, 8, 128], F32, tag="c")
        nc.vector.memset(c_sb, 0.0)

        def mm(out, stat, mov, start, stop, perf=None):
            nc.tensor.matmul(out, stat, mov, start=start, stop=stop,
                             perf_mode=perf, skip_group_check=True)

        def z_with_input(in_stat, close):
            """Allocate 4 pair z-tiles ([64, 2, 512], 2 PSUM banks each) and
            accumulate the input contribution (first MM into each bank)."""
            zs = []
            for p in range(4):
                z = zpool.tile([64, 2, 512], F32, tag="z")
                for jj in range(2):
                    nA = (2 * p + jj) * 512
                    mm(z[:, jj, :], in_stat, kb_sb[:, nA : nA + 512],
                       True, close)
                zs.append(z)
            return zs

        # step-0 z: h=0, so input-only (closed)
        zs = z_with_input(xt_sb[:, 0:b], close=True)

        hT_prev = None  # (fmt, tile): fmt "8" or "b"
        for t in range(NSTEPS):
            h_cur = hpool.tile([64, 8, 128], F32R, tag="h")

            if zs is None:
                # decode step (always in the bf16 region): rec MMs first,
                # input (pred, available only now) appended last
                zs = [zpool.tile([64, 2, 512], F32, tag="z", name="z")
                      for _ in range(4)]
                in_stat = dxt[:, (t - T) * b : (t - T + 1) * b]
                # jj innermost: both banks of a pair share each stationary load
                for p in range(4):
                    z = zs[p]
                    for k in range(8):
                        for jj in range(2):
                            nA = (2 * p + jj) * 512
                            mm(z[:, jj, :], hT_prev[1][:, k, :],
                               wb[:, k, nA : nA + 512], k == 0, False)
                    for jj in range(2):
                        nA = (2 * p + jj) * 512
                        mm(z[:, jj, :], in_stat, kb_sb[:, nA : nA + 512],
                           False, True)
            elif t > 0:
                # warmup: input contribution already accumulated; add the
                # recurrent part (fp8 DoubleRow or bf16 per region)
                if t < T_FP8:
                    # q-outer over pairs 0-2: one DR stationary load (the
                    # expensive 256-col load) serves 6 matmuls. Pair 3 is
                    # peeled last: under zpool bufs=3 its PSUM banks alias
                    # pair 0's, so its writes must follow pair 0's
                    # gate-math reads — q-outer including it would stall
                    # the in-order PE queue behind pair 0's own matmuls.
                    for q in range(4):
                        for p in range(3):
                            for jj in range(2):
                                nA = (2 * p + jj) * 512
                                mm(zs[p][:, jj, :],
                                   hT_prev[1][:, 2 * q : 2 * q + 2, :],
                                   w8[:, 2 * q : 2 * q + 2, nA : nA + 512],
                                   False, q == 3, perf=DR)
                    for q in range(4):
                        for jj in range(2):
                            nA = (6 + jj) * 512
                            mm(zs[3][:, jj, :],
                               hT_prev[1][:, 2 * q : 2 * q + 2, :],
                               w8[:, 2 * q : 2 * q + 2, nA : nA + 512],
                               False, q == 3, perf=DR)
                else:
                    # jj innermost: both banks share each stationary load
                    for p in range(4):
                        z = zs[p]
                        for k in range(8):
                            for jj in range(2):
                                nA = (2 * p + jj) * 512
                                mm(z[:, jj, :], hT_prev[1][:, k, :],
                                   wb[:, k, nA : nA + 512], False, k == 7)

            # gate math per pair; z cols: [i 0:128 | f 128:256 | o 256:384 | g 384:512]
            for p in range(4):
                z = zs[p]
                sfo = gpool.tile([64, 2, 384], F32, tag="sfo")
                nc.scalar.activation(sfo, z[:, :, 0:384], AF.Sigmoid,
                                     scale=1.0 / W_SCALE)
                gt = gpool.tile([64, 2, 128], F32, tag="gt")
                nc.scalar.activation(gt, z[:, :, 384:512], AF.Tanh,
                                     scale=1.0 / W_SCALE)
                t1 = gpool.tile([64, 2, 128], F32, tag="t1")
                nc.vector.tensor_mul(t1, sfo[:, :, 0:128], gt)
                cj = c_sb[:, 2 * p : 2 * p + 2, :]
                nc.vector.tensor_mul(cj, sfo[:, :, 128:256], cj)
                nc.vector.tensor_add(cj, cj, t1)
                tct = gpool.tile([64, 2, 128], F32, tag="tct")
                nc.scalar.activation(tct, cj, AF.Tanh)
                hj = h_cur[:, 2 * p : 2 * p + 2, :]
                nc.vector.tensor_mul(hj, sfo[:, :, 256:384], tct)

            # early input matmuls for the next warmup step (independent of
            # h): fills the PE during this step's gate-math tail
            if t + 1 < T:
                zs = z_with_input(xt_sb[:, (t + 1) * b : (t + 2) * b],
                                  close=False)
            else:
                zs = None

            # transpose h -> hT chunks; the PSUM->SBUF copy downcasts to the
            # dtype the NEXT step's matmuls need
            if t + 1 < T_FP8:
                hT_t = h8pool.tile([128, 8, 64], FP8, tag="h8")
                hT_cur = ("8", hT_t)
            else:
                hT_t = hbpool.tile([128, 8, 64], BF16, tag="hb")
                hT_cur = ("b", hT_t)
            for k in range(8):
                tp = tppool.tile([128, 64], F32R, tag="tp")
                nc.tensor.transpose(tp, h_cur[:, k, :], ident64)
                nc.vector.tensor_copy(hT_cur[1][:, k, :], tp)

            if t >= T - 1:
                d = t - (T - 1)
                ptt = zpool.tile([64, 2, 512], F32, tag="z")
                pt = ptt[0:4, 0, 0:64]
                for k in range(8):
                    mm(pt, dw_sb[:, 4 * k : 4 * k + 4], hT_cur[1][:, k, :],
                       k == 0, k == 7)
                nc.vector.tensor_scalar_add(
                    in_all[:, d * b : (d + 1) * b], pt, db_sb)
                if t + 1 < NSTEPS:
                    nc.vector.tensor_scalar_add(
                        dxt[0:4, d * b : (d + 1) * b], pt, db_sb)
            hT_prev = hT_cur

        nc.sync.dma_start(out=outp_d, in_=in_all)

        if rep_ctx is not None:
            rep_ctx.__exit__(None, None, None)

    nc.compile()  # bacc passes: wait-splitting (TRN2 allows 1 wait/inst), DCE
    return nc


def _prep_inputs(x, kern, rec_kernel, bias, dense_w, dense_b, S):
    """Host-side numpy prep: gate interleave, scaling, dtype casts, shards."""
    import ml_dtypes

    T, b = T_WARM, B_LOC
    f32 = np.float32
    FP8 = ml_dtypes.float8_e4m3
    BF16 = ml_dtypes.bfloat16
    # interleaved column order: per 128-unit slice j -> [i_j, f_j, o_j, g_j]
    perm = np.concatenate(
        [g * U + np.arange(128 * j, 128 * (j + 1))
         for j in range(8) for g in (0, 1, 3, 2)]
    )
    # rec weights, chunk-major [128, 8*4096], scaled x64
    wr = (rec_kernel[:, perm].reshape(8, 128, 4 * U).transpose(1, 0, 2)
          .reshape(128, 8 * 4 * U)) * W_SCALE
    w8 = wr.astype(FP8)

    CB = 8 * 4 * U + _cb_cols(S)
    cb = np.zeros((128, CB), f32)
    cb[:, 0 : 8 * 4 * U] = wr
    O = 8 * 4 * U
    cb[0:4, O + _KB0 : O + _KB0 + 4 * U] = kern[:, perm] * W_SCALE
    cb[4, O + _KB0 : O + _KB0 + 4 * U] = bias[perm] * W_SCALE
    cb[:, O + _DW0 : O + _DW0 + 32] = (
        dense_w.reshape(8, 128, NF).transpose(1, 0, 2).reshape(128, 32)
    )
    cb[4, O + _DX0 : O + _DX0 + S * b] = 1.0  # decode ones row

    cr = np.zeros((128, _cr_cols(S)), f32)
    cr[:, _ID0 : _ID0 + 128] = np.eye(128, dtype=f32)
    cr[0:4, _db_col(S)] = dense_b

    in_maps = []
    for m in range(N_CORES):
        cbm = cb.copy()
        xs = x[m * b : (m + 1) * b].astype(f32)  # [b, T, F]
        xT = xs.transpose(2, 1, 0).reshape(NF, T * b)  # col index = t*b + b_idx
        cbm[0:4, O + _XT0 : O + _XT0 + T * b] = xT
        cbm[4, O + _XT0 : O + _XT0 + T * b] = 1.0
        in_maps.append({
            "c8": np.ascontiguousarray(w8),
            "cb": np.ascontiguousarray(cbm.astype(BF16)),
            "cr": np.ascontiguousarray(cr),
        })
    return in_maps


def kernel(x, kernel, rec_kernel, bias, dense_w, dense_b, out_steps):
    from concourse import bass_utils

    S = int(out_steps)
    x = np.asarray(x, dtype=np.float32)
    nc = _build_program(S)
    in_maps = _prep_inputs(
        x, np.asarray(kernel, np.float32), np.asarray(rec_kernel, np.float32),
        np.asarray(bias, np.float32), np.asarray(dense_w, np.float32),
        np.asarray(dense_b, np.float32), S,
    )
    res = bass_utils.run_bass_kernel_spmd(
        nc, in_maps, core_ids=list(range(N_CORES)),
        trace=bool(int(os.environ.get("LSTM_KERNEL_TRACE", "0"))),
    )
    outs = []
    for m in range(N_CORES):
        o = res.results[m]["outp"]  # [4, S*b]
        outs.append(o.reshape(NF, S, B_LOC).transpose(2, 1, 0))  # [b, S, 4]
    return np.concatenate(outs, axis=0).astype(np.float32)  # [B, S, 4]


# revision 15
# speedup vs baseline: 1.1131x; 1.1131x over previous
"""Trainium2 Bass kernel for nn_AutoregressiveRegression (LSTM warmup + autoregressive decode).

Problem (per reference):
  B=512, T=128, F=4, U=1024, out_steps=32
  - warmup: LSTM over x[:, t, :] for t in 0..T-1 (h0=c0=0)
  - pred0 = h @ dense_w + dense_b
  - decode: 31 more LSTM steps feeding pred back as the input
  - output: [B, out_steps, 4]

Strategy (8 NeuronCores, data-parallel over batch, 64 rows/core):
  - All weights resident in SBUF; zero steady-state DMA, zero collectives.
  - The recurrence is PE-streaming-bound: each step streams the whole
    rec_kernel through the array as the moving operand (M=64 batch).
  - Hybrid precision, exploiting the LSTM forget-gate's geometric decay of
    injected noise (validated by numpy simulation; measured HW rel err
    4.0e-3 vs the 2e-2 gate):
      * steps t < NSTEPS-K_EXACT: fp8e4(e4m3) weights+h with DoubleRow
        matmuls (K=256 contraction per pass, 32 MMs/step vs 64).
      * last K_EXACT=40 steps (incl. all decode): bf16 weights+h; the
        bf16 noise floor dominates the final error.
    (On this walrus build DoubleRow does not reach its nominal 2x moving-
    operand rate - fp8 steps measure near bf16-step speed - but the halved
    matmul count plus shared-stationary pairing still wins. Measured
    2.46 ms total vs 2.80 ms baseline, repetition-slope method.)
  - Matmul emission order (pair, k, jj-bank) keeps consecutive matmuls on
    the same stationary operand, amortizing PE weight loads; the next
    step's input matmuls are issued early to fill the gate-math tail.
  - Weights are pre-scaled x64 host-side (fp8 dynamic-range centering);
    the descale is folded into the activation instructions (scale=1/64).
  - Weight columns gate-interleaved host-side per 128-unit slice j:
    [i_j | f_j | o_j | g_j]; j-chunk PAIRS share one [64, 2, 512] PSUM
    tile (2 banks) so all gate math runs as 3D-AP instructions covering
    both chunks -> half the ACT/DVE instruction overhead.
  - h is re-transposed each step on the PE; the PSUM->SBUF evacuation
    doubles as the fp8/bf16 downcast for the next step's stationary.
  - Decode feeds pred back via a bf16 staging block; the f32 copy of
    pred is the output staging.
"""

import os
from contextlib import ExitStack

import numpy as np

B_FULL = 512
T_WARM = 128
N_CORES = 8
B_LOC = B_FULL // N_CORES  # 64
U = 1024
NF = 4
K_EXACT = 40
W_SCALE = 64.0

# bf16 constant block column layout (elements per partition)
_KB0 = 0                      # kernel+bias rows 0:5: [5, 4096]
_XT0 = _KB0 + 4 * U           # x^T + ones row: [5, T*b]
_DW0 = _XT0 + T_WARM * B_LOC  # dense_w chunk-major: [128, 32]
_DX0 = _DW0 + 32              # decode input staging: [5, S*b] (row 4 = ones)


def _cb_cols(S):
    return _DX0 + S * B_LOC


# f32r constant block layout
_ID0 = 0                      # identity [128, 128]
_IA0 = _ID0 + 128             # output staging [4, S*b]


def _db_col(S):
    return _IA0 + S * B_LOC


def _cr_cols(S):
    return _db_col(S) + 1


def _build_program(S, reps=1):
    """Build the per-core Bass program (identical on all cores; data differs).

    reps > 1 wraps the whole computation (including load DMAs) in a hardware
    For_i loop — used only for timing (slope over reps isolates on-device
    exec time from the axon RPC noise)."""
    import concourse.mybir as mybir
    import concourse.tile as tile
    from concourse import bacc

    F32 = mybir.dt.float32
    F32R = mybir.dt.float32r
    BF16 = mybir.dt.bfloat16
    FP8 = mybir.dt.float8e4
    AF = mybir.ActivationFunctionType
    DR = mybir.MatmulPerfMode.DoubleRow

    T = T_WARM
    b = B_LOC
    NSTEPS = T + S - 1       # 159 recurrent steps
    T_FP8 = NSTEPS - K_EXACT  # steps t < T_FP8 use fp8 DoubleRow

    nc = bacc.Bacc("TRN2", target_bir_lowering=False, debug=False)

    c8_d = nc.dram_tensor("c8", [128, 8 * 4 * U], FP8, kind="ExternalInput").ap()
    cb_d = nc.dram_tensor("cb", [128, 8 * 4 * U + _cb_cols(S)], BF16,
                          kind="ExternalInput").ap()
    cr_d = nc.dram_tensor("cr", [128, _cr_cols(S)], F32R,
                          kind="ExternalInput").ap()
    outp_d = nc.dram_tensor("outp", [4, S * b], F32R, kind="ExternalOutput").ap()

    with tile.TileContext(nc) as tc, ExitStack() as ctx:
        singles = ctx.enter_context(tc.tile_pool(name="singles", bufs=1))
        h8pool = ctx.enter_context(tc.tile_pool(name="h8pool", bufs=2))
        hbpool = ctx.enter_context(tc.tile_pool(name="hbpool", bufs=2))
        hpool = ctx.enter_context(tc.tile_pool(name="hpool", bufs=2))
        gpool = ctx.enter_context(tc.tile_pool(name="gpool", bufs=2))
        zpool = ctx.enter_context(tc.tile_pool(name="zpool", bufs=3, space="PSUM"))
        tppool = ctx.enter_context(tc.tile_pool(name="tppool", bufs=2, space="PSUM"))

        rep_ctx = tc.For_i(0, reps, 1) if reps > 1 else None
        if rep_ctx is not None:
            rep_ctx.__enter__()

        # DMA order matters: cstb (x/kernel, needed at t=0) and the fp8
        # weights (needed at t=1) first; the bf16 weights are not consumed
        # until step T_FP8 (~1.1 ms in), so their 8 MB load hides behind
        # the fp8-region compute.
        w8 = singles.tile([128, 8, 4 * U], FP8, tag="w8")
        wb = singles.tile([128, 8, 4 * U], BF16, tag="wb")
        cstb = singles.tile([128, _cb_cols(S)], BF16, tag="cstb")
        nc.sync.dma_start(out=cstb, in_=cb_d[:, 8 * 4 * U :])
        cstr = singles.tile([128, _cr_cols(S)], F32R, tag="cstr")
        nc.sync.dma_start(out=cstr, in_=cr_d)
        for k in range(8):
            nc.sync.dma_start(out=w8[:, k, :],
                              in_=c8_d[:, k * 4 * U : (k + 1) * 4 * U])
        for k in range(8):
            nc.sync.dma_start(out=wb[:, k, :],
                              in_=cb_d[:, k * 4 * U : (k + 1) * 4 * U])

        # input operands K-padded to 128 rows (rows 5:128 are zeros
        # host-side): the input matmuls then run in the same (128, 64)
        # tile mode as the bf16 recurrent matmuls — no PE mode-switch
        # drain around the input block
        kb_sb = cstb[0:128, _KB0 : _KB0 + 4 * U]
        xt_sb = cstb[0:128, _XT0 : _XT0 + T * b]
        dw_sb = cstb[:, _DW0 : _DW0 + 32]
        dxt = cstb[0:128, _DX0 : _DX0 + S * b]
        ident64 = cstr[0:64, _ID0 : _ID0 + 64]
        in_all = cstr[0:4, _IA0 : _IA0 + S * b]
        db_sb = singles.tile([4, 1], F32, tag="db")
        nc.gpsimd.dma_start(out=db_sb, in_=cr_d[0:4, _db_col(S) : _db_col(S) + 1])

        c_sb = singles.tile([64, 8, 128], F32, tag="c")
        nc.vector.memset(c_sb, 0.0)

        def mm(out, stat, mov, start, stop, perf=None):
            nc.tensor.matmul(out, stat, mov, start=start, stop=stop,
                             perf_mode=perf, skip_group_check=True)

        def z_with_input(in_stat, close):
            """Allocate 4 pair z-tiles ([64, 2, 512], 2 PSUM banks each) and
            accumulate the input contribution (first MM into each bank)."""
            zs = []
            for p in range(4):
                z = zpool.tile([64, 2, 512], F32, tag="z")
                for jj in range(2):
                    nA = (2 * p + jj) * 512
                    mm(z[:, jj, :], in_stat, kb_sb[:, nA : nA + 512],
                       True, close)
                zs.append(z)
            return zs

        # step-0 z: h=0, so input-only (closed)
        zs = z_with_input(xt_sb[:, 0:b], close=True)

        hT_prev = None  # (fmt, tile): fmt "8" or "b"
        for t in range(NSTEPS):
            h_cur = hpool.tile([64, 8, 128], F32R, tag="h")

            if zs is None:
                # decode step (always in the bf16 region): rec MMs first,
                # input (pred, available only now) appended last
                zs = [zpool.tile([64, 2, 512], F32, tag="z", name="z")
                      for _ in range(4)]
                in_stat = dxt[:, (t - T) * b : (t - T + 1) * b]
                # jj innermost: both banks of a pair share each stationary load
                for p in range(4):
                    z = zs[p]
                    for k in range(8):
                        for jj in range(2):
                            nA = (2 * p + jj) * 512
                            mm(z[:, jj, :], hT_prev[1][:, k, :],
                               wb[:, k, nA : nA + 512], k == 0, False)
                    for jj in range(2):
                        nA = (2 * p + jj) * 512
                        mm(z[:, jj, :], in_stat, kb_sb[:, nA : nA + 512],
                           False, True)
            elif t > 0:
                # warmup: input contribution already accumulated; add the
                # recurrent part (fp8 DoubleRow or bf16 per region)
                # jj innermost: both banks of a pair share each stationary
                # load, and each pair's z closes early so gate math
                # pipelines under the remaining pairs' matmuls. (A q-outer
                # schedule that amortizes the DR stationary load across all
                # pairs measured SLOWER — 2.61 ms vs 2.41 ms — the late
                # bank-close serializes the gate-math tail.)
                for p in range(4):
                    z = zs[p]
                    if t < T_FP8:
                        for q in range(4):
                            for jj in range(2):
                                nA = (2 * p + jj) * 512
                                mm(z[:, jj, :], hT_prev[1][:, 2 * q : 2 * q + 2, :],
                                   w8[:, 2 * q : 2 * q + 2, nA : nA + 512],
                                   False, q == 3, perf=DR)
                    else:
                        for k in range(8):
                            for jj in range(2):
                                nA = (2 * p + jj) * 512
                                mm(z[:, jj, :], hT_prev[1][:, k, :],
                                   wb[:, k, nA : nA + 512], False, k == 7)

            # gate math per pair; z cols: [i 0:128 | f 128:256 | o 256:384 | g 384:512]
            for p in range(4):
                z = zs[p]
                sfo = gpool.tile([64, 2, 384], F32, tag="sfo")
                nc.scalar.activation(sfo, z[:, :, 0:384], AF.Sigmoid,
                                     scale=1.0 / W_SCALE)
                gt = gpool.tile([64, 2, 128], F32, tag="gt")
                nc.scalar.activation(gt, z[:, :, 384:512], AF.Tanh,
                                     scale=1.0 / W_SCALE)
                t1 = gpool.tile([64, 2, 128], F32, tag="t1")
                nc.vector.tensor_mul(t1, sfo[:, :, 0:128], gt)
                cj = c_sb[:, 2 * p : 2 * p + 2, :]
                nc.vector.tensor_mul(cj, sfo[:, :, 128:256], cj)
                nc.vector.tensor_add(cj, cj, t1)
                tct = gpool.tile([64, 2, 128], F32, tag="tct")
                nc.scalar.activation(tct, cj, AF.Tanh)
                hj = h_cur[:, 2 * p : 2 * p + 2, :]
                nc.vector.tensor_mul(hj, sfo[:, :, 256:384], tct)

            # early input matmuls for the next warmup step (independent of
            # h): fills the PE during this step's gate-math tail
            if t + 1 < T:
                zs = z_with_input(xt_sb[:, (t + 1) * b : (t + 2) * b],
                                  close=False)
            else:
                zs = None

            # transpose h -> hT chunks; the PSUM->SBUF copy downcasts to the
            # dtype the NEXT step's matmuls need
            if t + 1 < T_FP8:
                hT_t = h8pool.tile([128, 8, 64], FP8, tag="h8")
                hT_cur = ("8", hT_t)
            else:
                hT_t = hbpool.tile([128, 8, 64], BF16, tag="hb")
                hT_cur = ("b", hT_t)
            for k in range(8):
                tp = tppool.tile([128, 64], F32R, tag="tp")
                nc.tensor.transpose(tp, h_cur[:, k, :], ident64)
                nc.vector.tensor_copy(hT_cur[1][:, k, :], tp)

            if t >= T - 1:
                d = t - (T - 1)
                ptt = zpool.tile([64, 2, 512], F32, tag="z")
                pt = ptt[0:4, 0, 0:64]
                for k in range(8):
                    mm(pt, dw_sb[:, 4 * k : 4 * k + 4], hT_cur[1][:, k, :],
                       k == 0, k == 7)
                nc.vector.tensor_scalar_add(
                    in_all[:, d * b : (d + 1) * b], pt, db_sb)
                if t + 1 < NSTEPS:
                    nc.vector.tensor_scalar_add(
                        dxt[0:4, d * b : (d + 1) * b], pt, db_sb)
            hT_prev = hT_cur

        nc.sync.dma_start(out=outp_d, in_=in_all)

        if rep_ctx is not None:
            rep_ctx.__exit__(None, None, None)

    nc.compile()  # bacc passes: wait-splitting (TRN2 allows 1 wait/inst), DCE
    return nc


def _prep_inputs(x, kern, rec_kernel, bias, dense_w, dense_b, S):
    """Host-side numpy prep: gate interleave, scaling, dtype casts, shards."""
    import ml_dtypes

    T, b = T_WARM, B_LOC
    f32 = np.float32
    FP8 = ml_dtypes.float8_e4m3
    BF16 = ml_dtypes.bfloat16
    # interleaved column order: per 128-unit slice j -> [i_j, f_j, o_j, g_j]
    perm = np.concatenate(
        [g * U + np.arange(128 * j, 128 * (j + 1))
         for j in range(8) for g in (0, 1, 3, 2)]
    )
    # rec weights, chunk-major [128, 8*4096], scaled x64
    wr = (rec_kernel[:, perm].reshape(8, 128, 4 * U).transpose(1, 0, 2)
          .reshape(128, 8 * 4 * U)) * W_SCALE
    w8 = wr.astype(FP8)

    CB = 8 * 4 * U + _cb_cols(S)
    cb = np.zeros((128, CB), f32)
    cb[:, 0 : 8 * 4 * U] = wr
    O = 8 * 4 * U
    cb[0:4, O + _KB0 : O + _KB0 + 4 * U] = kern[:, perm] * W_SCALE
    cb[4, O + _KB0 : O + _KB0 + 4 * U] = bias[perm] * W_SCALE
    cb[:, O + _DW0 : O + _DW0 + 32] = (
        dense_w.reshape(8, 128, NF).transpose(1, 0, 2).reshape(128, 32)
    )
    cb[4, O + _DX0 : O + _DX0 + S * b] = 1.0  # decode ones row

    cr = np.zeros((128, _cr_cols(S)), f32)
    cr[:, _ID0 : _ID0 + 128] = np.eye(128, dtype=f32)
    cr[0:4, _db_col(S)] = dense_b

    in_maps = []
    for m in range(N_CORES):
        cbm = cb.copy()
        xs = x[m * b : (m + 1) * b].astype(f32)  # [b, T, F]
        xT = xs.transpose(2, 1, 0).reshape(NF, T * b)  # col index = t*b + b_idx
        cbm[0:4, O + _XT0 : O + _XT0 + T * b] = xT
        cbm[4, O + _XT0 : O + _XT0 + T * b] = 1.0
        in_maps.append({
            "c8": np.ascontiguousarray(w8),
            "cb": np.ascontiguousarray(cbm.astype(BF16)),
            "cr": np.ascontiguousarray(cr),
        })
    return in_maps


def kernel(x, kernel, rec_kernel, bias, dense_w, dense_b, out_steps):
    from concourse import bass_utils

    S = int(out_steps)
    x = np.asarray(x, dtype=np.float32)
    nc = _build_program(S)
    in_maps = _prep_inputs(
        x, np.asarray(kernel, np.float32), np.asarray(rec_kernel, np.float32),
        np.asarray(bias, np.float32), np.asarray(dense_w, np.float32),
        np.asarray(dense_b, np.float32), S,
    )
    res = bass_utils.run_bass_kernel_spmd(
        nc, in_maps, core_ids=list(range(N_CORES)),
        trace=bool(int(os.environ.get("LSTM_KERNEL_TRACE", "0"))),
    )
    outs = []
    for m in range(N_CORES):
        o = res.results[m]["outp"]  # [4, S*b]
        outs.append(o.reshape(NF, S, B_LOC).transpose(2, 1, 0))  # [b, S, 4]
    return np.concatenate(outs, axis=0).astype(np.float32)  # [B, S, 4]
